# revision 38
# baseline (speedup 1.0000x reference)
"""CliffordAttention Trainium2 kernel.

Math (see reference): per (b, h):
    q = x @ Wq.T + bq ; k = x @ Wk.T + bk ; v = x @ Wv.T + bv   (head h owns
    128 contiguous channels of the 1024 output features)
    S[l, m]  = sum_c (q[l,c] * sign_c * scale) * k[m,c]
    attn     = softmax_m(S)
    out[l,c] = sum_m attn[l, m] v[m, c]
Outputs: out (B, L, 1024) and attn.mean over heads (B, L, L).

Sharding: 8 cores = 2 batches x 4 head-pairs. Each core computes its two
heads end-to-end; host sums the per-head attention partials (4 cores x 2
heads per batch), transposes, and divides by 8.

On-device layout: everything transposed (channels on partitions):
    qwT/kT: [c=128, l=2048] per head (sign*scale folded into Wq on host)
    v:      [l=128-tiles, c=256] natural (used as AV lhsT)
    S^T:    [m, l] per (head, l-half) -> exp on ACT -> bf16
    AV:     psum[c, l] = sum_m v[m,c] * expS^T[m,l]
    denom:  psum[1, l] = sum_m expS^T[m,l]   (ones-matmul)
    attn^T: expS^T * bcast(1/denom)  (DVE), written [m, l]; host transposes.
    out:    (psum_av * bcast(1/denom)) -> PE transpose -> [l, c] -> HBM
"""

import math
from contextlib import ExitStack

import numpy as np
import ml_dtypes

import concourse.bass as bass
import concourse.bacc as bacc
import concourse.mybir as mybir
import concourse.tile as tile

B, L, H, D, NB = 2, 2048, 8, 16, 8
DM = H * D * NB  # 1024
P = 128
HPC = 2  # heads per core
CPC = HPC * P  # channels per core (256)
NCORES = 8
KT = DM // P  # 8 K-tiles for projections
MT = L // P  # 16 m-tiles
LH = 2  # l-halves
LHS = L // LH  # 1024

BF16 = mybir.dt.bfloat16
F32 = mybir.dt.float32

REV_SIGNS = np.array([1, 1, 1, 1, -1, -1, -1, -1], dtype=np.float32)


def build_program() -> bass.Bass:
    nc = bacc.Bacc(None, target_bir_lowering=False)

    xT = nc.dram_tensor("xT", [DM, L], BF16, kind="ExternalInput")
    wAll = nc.dram_tensor("wAll", [DM, 3 * CPC], BF16, kind="ExternalInput")
    bqk = nc.dram_tensor("bqk", [P, 4], F32, kind="ExternalInput")
    bvb = nc.dram_tensor("bvb", [P, CPC], F32, kind="ExternalInput")

    attnT = nc.dram_tensor("attnT", [HPC, L, L], BF16, kind="ExternalOutput")
    outpT = nc.dram_tensor("outpT", [HPC, P, L], F32, kind="ExternalOutput")

    with tile.TileContext(nc) as tc, ExitStack() as ctx:
        const = ctx.enter_context(tc.tile_pool(name="const", bufs=1))
        sb = ctx.enter_context(tc.tile_pool(name="sb", bufs=3))
        pexp = ctx.enter_context(tc.tile_pool(name="pexp", bufs=32))
        pattn = ctx.enter_context(tc.tile_pool(name="pattn", bufs=10))
        # PSUM budget is 8 banks of 2KB:
        #   s-tag  [128,1024]f32 x2 bufs = 4 banks (S^T fills, projections,
        #                                           out transposes)
        #   av-tag [128,1024]f32 x1      = 2 banks (AV accumulator, v proj)
        #   d-tag  [1,1024]f32   x1      = 2 banks (denominator row)
        pss = ctx.enter_context(tc.tile_pool(name="pss", bufs=2, space="PSUM"))
        pav = ctx.enter_context(tc.tile_pool(name="pav", bufs=1, space="PSUM"))
        pden = ctx.enter_context(tc.tile_pool(name="pden", bufs=1, space="PSUM"))
        pdram = ctx.enter_context(tc.tile_pool(name="pdram", bufs=2, space="DRAM"))

        # ---- constants -------------------------------------------------
        ones_col = const.tile([P, 1], BF16, tag="ones")
        nc.vector.memset(ones_col, 1.0)
        # warm the tensor engine clock while input DMAs stream in
        scratch = const.tile([P, 512], BF16, tag="scratch")
        nc.vector.memset(scratch, 0.0)
        warm = pden.tile([1, 512], F32, tag="d")
        for _ in range(30):
            nc.tensor.matmul(warm, lhsT=ones_col, rhs=scratch, start=True, stop=True)
        # ---- load inputs: one DMA per K-tile for x and for packed W ----
        xt = []
        wts = [[], [], []]  # wts[p][k] : [128, 256] views into packed tiles
        for k in range(KT):
            tw = const.tile([P, 3 * CPC], BF16, tag=f"w{k}")
            nc.sync.dma_start(out=tw, in_=wAll[k * P : (k + 1) * P, :])
            for pi in range(3):
                wts[pi].append(tw[:, pi * CPC : (pi + 1) * CPC])
            t = const.tile([P, L], BF16, tag=f"xt{k}")
            nc.sync.dma_start(out=t, in_=xT[k * P : (k + 1) * P, :])
            xt.append(t)

        bqk_sb = const.tile([P, 4], F32, tag="bqk")
        nc.sync.dma_start(out=bqk_sb, in_=bqk[:, :])
        bvb_sb = const.tile([P, CPC], F32, tag="bvb")
        nc.sync.dma_start(out=bvb_sb, in_=bvb[:, :])

        # ---- projections ----------------------------------------------
        # q and k in transposed layout: [c(128) x l(2048)] per head.
        qwT = [None, None]
        kTt = [None, None]

        def project_qk(pi, ct):
            dst = const.tile([P, L], BF16, tag=f"p{pi}_{ct}")
            for half in range(LH):
                ps = pss.tile([P, LHS], F32, tag="s")
                for j in range(LHS // 512):
                    lo = half * LHS + j * 512
                    for k in range(KT):
                        nc.tensor.matmul(
                            ps[:, j * 512 : (j + 1) * 512],
                            lhsT=wts[pi][k][:, ct * P : (ct + 1) * P],
                            rhs=xt[k][:, lo : lo + 512],
                            start=(k == 0),
                            stop=(k == KT - 1),
                        )
                nc.scalar.activation(
                    dst[:, half * LHS : (half + 1) * LHS],
                    ps,
                    mybir.ActivationFunctionType.Identity,
                    bias=bqk_sb[:, 2 * pi + ct : 2 * pi + ct + 1],
                    scale=1.0,
                )
            (qwT if pi == 0 else kTt)[ct] = dst

        project_qk(0, 0)
        project_qk(0, 1)
        project_qk(1, 0)
        project_qk(1, 1)

        # v in natural layout: 16 tiles [l(128) x c(256)].
        v_sb = []
        for g in range(4):  # groups of 4 l-tiles share one psum fill
            ps = pss.tile([P, LHS], F32, tag="s")
            for li in range(4):
                lt = g * 4 + li
                for k in range(KT):
                    nc.tensor.matmul(
                        ps[:, li * CPC : (li + 1) * CPC],
                        lhsT=xt[k][:, lt * P : (lt + 1) * P],
                        rhs=wts[2][k][:, :],
                        start=(k == 0),
                        stop=(k == KT - 1),
                    )
            for li in range(4):
                lt = g * 4 + li
                t = const.tile([P, CPC], BF16, tag=f"v{lt}")
                nc.vector.tensor_add(
                    out=t, in0=ps[:, li * CPC : (li + 1) * CPC], in1=bvb_sb
                )
                v_sb.append(t)

        # ---- attention -------------------------------------------------
        def attention_chunk(h, lo, width):
            """One head's attention for query columns [lo, lo+width)."""
            nj = width // 512
            ps_av = pav.tile([P, width], F32, tag="av")
            ps_d = pden.tile([1, width], F32, tag="d")
            exps = []
            for mt in range(MT):
                ps_s = pss.tile([P, width], F32, tag="s")
                for j in range(nj):
                    nc.tensor.matmul(
                        ps_s[:, j * 512 : (j + 1) * 512],
                        lhsT=kTt[h][:, mt * P : (mt + 1) * P],
                        rhs=qwT[h][:, lo + j * 512 : lo + (j + 1) * 512],
                        start=True,
                        stop=True,
                    )
                e = pexp.tile([P, width], BF16, tag="exps")
                nc.scalar.activation(e, ps_s, mybir.ActivationFunctionType.Exp)
                exps.append(e)
                for j in range(nj):
                    sl = slice(j * 512, (j + 1) * 512)
                    nc.tensor.matmul(
                        ps_av[:, sl],
                        lhsT=v_sb[mt][:, h * P : (h + 1) * P],
                        rhs=e[:, sl],
                        start=(mt == 0),
                        stop=(mt == MT - 1),
                    )
                    nc.tensor.matmul(
                        ps_d[:, sl],
                        lhsT=ones_col,
                        rhs=e[:, sl],
                        start=(mt == 0),
                        stop=(mt == MT - 1),
                    )

            # evict AV accumulator early so the next chunk's psum slot
            # frees without waiting on the reciprocal chain
            avraw = sb.tile([P, width], F32, tag="avraw")
            nc.vector.tensor_copy(out=avraw, in_=ps_av)

            # reciprocal of denominator: bounce the [1, width] row through
            # DRAM into a [128, width/128] layout so DVE's 8-cycle/elem
            # divide runs on 128 partitions instead of one
            drow = sb.tile([1, width], F32, tag="drow")
            nc.vector.tensor_copy(out=drow, in_=ps_d)
            den_d = pdram.tile([1, width], F32, tag="den_d")
            nc.sync.dma_start(out=den_d, in_=drow)
            w_p = width // P
            dcol = sb.tile([P, w_p], F32, tag="dcol")
            nc.sync.dma_start(
                out=dcol,
                in_=bass.AP(
                    tensor=den_d.tensor,
                    offset=den_d.offset,
                    ap=[[w_p, P], [1, w_p]],
                ),
            )
            rcol = sb.tile([P, w_p], F32, tag="rcol")
            nc.vector.reciprocal(rcol, dcol)
            rcol16 = sb.tile([P, w_p], BF16, tag="rcol16")
            nc.vector.tensor_copy(out=rcol16, in_=rcol)
            recip_d = pdram.tile([1, width], BF16, tag="recip_d")
            nc.sync.dma_start(
                out=bass.AP(
                    tensor=recip_d.tensor,
                    offset=recip_d.offset,
                    ap=[[w_p, P], [1, w_p]],
                ),
                in_=rcol16,
            )
            rb16 = sb.tile([P, width], BF16, tag="rb16")
            nc.sync.dma_start(
                out=rb16,
                in_=bass.AP(
                    tensor=recip_d.tensor,
                    offset=recip_d.offset,
                    ap=[[0, P], [1, width]],
                ),
            )

            # out: normalize and write [c, l] (host transposes)
            outT = sb.tile([P, width], F32, tag="outT")
            nc.vector.tensor_mul(out=outT, in0=avraw, in1=rb16)
            nc.sync.dma_start(out=outpT[h, :, lo : lo + width], in_=outT)

            # attention probabilities: normalize and write [m, l] slab
            for mt in range(MT):
                a = pattn.tile([P, width], BF16, tag="attn")
                nc.vector.tensor_mul(out=a, in0=exps[mt], in1=rb16)
                eng = nc.gpsimd if mt % 2 else nc.sync
                eng.dma_start(
                    out=attnT[h, mt * P : (mt + 1) * P, lo : lo + width],
                    in_=a,
                )

        attention_chunk(0, 0, LHS)
        attention_chunk(1, 0, LHS)
        attention_chunk(0, LHS, LHS)
        attention_chunk(1, LHS, LHS)
    nc.finalize()
    return nc


_PROGRAM_CACHE: dict[str, bass.Bass] = {}


def _get_program() -> bass.Bass:
    if "nc" not in _PROGRAM_CACHE:
        _PROGRAM_CACHE["nc"] = build_program()
    return _PROGRAM_CACHE["nc"]


def make_in_maps(query, Wq, bq, Wk, bk, Wv, bv):
    """Build the 8 per-core input dicts (host-side shard prep)."""
    scale = 1.0 / math.sqrt(D * NB)
    signs = np.tile(REV_SIGNS, DM // NB).astype(np.float32)  # per channel
    bf = ml_dtypes.bfloat16

    in_maps = []
    for c in range(NCORES):
        b = c // 4
        cs = (c % 4) * CPC  # channel start (head-pair offset)
        sl = slice(cs, cs + CPC)
        sgn = (signs[sl] * scale)[:, None]  # (256, 1)

        xT = np.ascontiguousarray(query[b].T).astype(bf)  # (1024, 2048)
        wq = (Wq[sl, :] * sgn).T
        wk = Wk[sl, :].T
        wv = Wv[sl, :].T
        wall = np.ascontiguousarray(
            np.concatenate([wq, wk, wv], axis=1)
        ).astype(bf)  # (1024, 768)
        bq_f = (bq[sl] * sgn[:, 0]).astype(np.float32)
        bk_f = bk[sl].astype(np.float32)
        bqk_arr = np.stack(
            [bq_f[:P], bq_f[P:], bk_f[:P], bk_f[P:]], axis=1
        ).astype(np.float32)  # (128, 4)
        bvb_arr = np.broadcast_to(bv[sl].astype(np.float32), (P, CPC)).copy()

        in_maps.append(dict(xT=xT, wAll=wall, bqk=bqk_arr, bvb=bvb_arr))
    return in_maps


def assemble(results):
    """Gather per-core outputs into (out, attn_mean)."""
    out = np.zeros((B, L, DM), dtype=np.float32)
    attn_sum = np.zeros((B, L, L), dtype=np.float32)
    for c in range(NCORES):
        b = c // 4
        cs = (c % 4) * CPC
        r = results[c]
        ot = np.asarray(r["outpT"], dtype=np.float32)  # (2, 128, L)
        out[b, :, cs : cs + P] = ot[0].T
        out[b, :, cs + P : cs + CPC] = ot[1].T
        at = np.asarray(r["attnT"], dtype=np.float32)  # (2, L(m), L(l))
        attn_sum[b] += at[0].T
        attn_sum[b] += at[1].T
    attn_mean = attn_sum / float(H)
    return out, attn_mean


def run_on_device(in_maps, trace=False):
    from concourse.bass_utils import run_bass_kernel_spmd

    nc = _get_program()
    res = run_bass_kernel_spmd(
        nc, in_maps, core_ids=list(range(NCORES)), trace=trace
    )
    return res


def kernel(query, Wq, bq, Wk, bk, Wv, bv):
    query = np.asarray(query, dtype=np.float32)
    in_maps = make_in_maps(
        query,
        np.asarray(Wq, np.float32),
        np.asarray(bq, np.float32),
        np.asarray(Wk, np.float32),
        np.asarray(bk, np.float32),
        np.asarray(Wv, np.float32),
        np.asarray(bv, np.float32),
    )
    res = run_on_device(in_maps)
    return assemble(res.results)


# revision 39
# speedup vs baseline: 1.0161x; 1.0161x over previous
"""CliffordAttention Trainium2 kernel.

Math (see reference): per (b, h):
    q = x @ Wq.T + bq ; k = x @ Wk.T + bk ; v = x @ Wv.T + bv   (head h owns
    128 contiguous channels of the 1024 output features)
    S[l, m]  = sum_c (q[l,c] * sign_c * scale) * k[m,c]
    attn     = softmax_m(S)
    out[l,c] = sum_m attn[l, m] v[m, c]
Outputs: out (B, L, 1024) and attn.mean over heads (B, L, L).

Sharding: 8 cores = 2 batches x 4 head-pairs. Each core computes its two
heads end-to-end; host sums the per-head attention partials (4 cores x 2
heads per batch), transposes, and divides by 8.

On-device layout: everything transposed (channels on partitions):
    qwT/kT: [c=128, l=2048] per head (sign*scale folded into Wq on host)
    v:      [l=128-tiles, c=256] natural (used as AV lhsT)
    S^T:    [m, l] per (head, l-half) -> exp on ACT -> bf16
    AV:     psum[c, l] = sum_m v[m,c] * expS^T[m,l]
    denom:  psum[1, l] = sum_m expS^T[m,l]   (ones-matmul)
    attn^T: expS^T * bcast(1/denom)  (DVE), written [m, l]; host transposes.
    out:    (psum_av * bcast(1/denom)) -> PE transpose -> [l, c] -> HBM
"""

import math
from contextlib import ExitStack

import numpy as np
import ml_dtypes

import concourse.bass as bass
import concourse.bacc as bacc
import concourse.mybir as mybir
import concourse.tile as tile

B, L, H, D, NB = 2, 2048, 8, 16, 8
DM = H * D * NB  # 1024
P = 128
HPC = 2  # heads per core
CPC = HPC * P  # channels per core (256)
NCORES = 8
KT = DM // P  # 8 K-tiles for projections
MT = L // P  # 16 m-tiles
LH = 2  # l-halves
LHS = L // LH  # 1024

BF16 = mybir.dt.bfloat16
F32 = mybir.dt.float32

REV_SIGNS = np.array([1, 1, 1, 1, -1, -1, -1, -1], dtype=np.float32)


def build_program() -> bass.Bass:
    nc = bacc.Bacc(None, target_bir_lowering=False)

    xT = nc.dram_tensor("xT", [DM, L], BF16, kind="ExternalInput")
    wAll = nc.dram_tensor("wAll", [DM, 3 * CPC], BF16, kind="ExternalInput")
    bqk = nc.dram_tensor("bqk", [P, 4], F32, kind="ExternalInput")
    bvb = nc.dram_tensor("bvb", [P, CPC], F32, kind="ExternalInput")

    attnT = nc.dram_tensor("attnT", [HPC, L, L], BF16, kind="ExternalOutput")
    outpT = nc.dram_tensor("outpT", [HPC, P, L], F32, kind="ExternalOutput")

    with tile.TileContext(nc) as tc, ExitStack() as ctx:
        const = ctx.enter_context(tc.tile_pool(name="const", bufs=1))
        sb = ctx.enter_context(tc.tile_pool(name="sb", bufs=2))
        pexp = ctx.enter_context(tc.tile_pool(name="pexp", bufs=32))
        pattn = ctx.enter_context(tc.tile_pool(name="pattn", bufs=10))
        # PSUM budget is 8 banks of 2KB:
        #   s-tag  [128,1024]f32 x2 bufs = 4 banks (S^T fills, projections,
        #                                           out transposes)
        #   av-tag [128,1024]f32 x1      = 2 banks (AV accumulator, v proj)
        #   d-tag  [1,1024]f32   x1      = 2 banks (denominator row)
        pss = ctx.enter_context(tc.tile_pool(name="pss", bufs=2, space="PSUM"))
        pav = ctx.enter_context(tc.tile_pool(name="pav", bufs=1, space="PSUM"))
        pden = ctx.enter_context(tc.tile_pool(name="pden", bufs=1, space="PSUM"))
        pdram = ctx.enter_context(tc.tile_pool(name="pdram", bufs=2, space="DRAM"))

        # ---- constants -------------------------------------------------
        ones_col = const.tile([P, 1], BF16, tag="ones")
        nc.vector.memset(ones_col, 1.0)
        # warm the tensor engine clock while input DMAs stream in
        scratch = const.tile([P, 512], BF16, tag="scratch")
        nc.vector.memset(scratch, 0.0)
        warm = pden.tile([1, 512], F32, tag="d")
        for _ in range(30):
            nc.tensor.matmul(warm, lhsT=ones_col, rhs=scratch, start=True, stop=True)
        # ---- load inputs: one DMA per K-tile for x and for packed W ----
        xt = []
        wts = [[], [], []]  # wts[p][k] : [128, 256] views into packed tiles
        for k in range(KT):
            tw = const.tile([P, 3 * CPC], BF16, tag=f"w{k}")
            nc.sync.dma_start(out=tw, in_=wAll[k * P : (k + 1) * P, :])
            for pi in range(3):
                wts[pi].append(tw[:, pi * CPC : (pi + 1) * CPC])
            t = const.tile([P, L], BF16, tag=f"xt{k}")
            nc.sync.dma_start(out=t, in_=xT[k * P : (k + 1) * P, :])
            xt.append(t)

        bqk_sb = const.tile([P, 4], F32, tag="bqk")
        nc.sync.dma_start(out=bqk_sb, in_=bqk[:, :])
        bvb_sb = const.tile([P, CPC], F32, tag="bvb")
        nc.sync.dma_start(out=bvb_sb, in_=bvb[:, :])

        # ---- projections ----------------------------------------------
        # q and k in transposed layout: [c(128) x l(2048)] per head.
        qwT = [None, None]
        kTt = [None, None]

        def project_qk(pi, ct):
            dst = const.tile([P, L], BF16, tag=f"p{pi}_{ct}")
            for half in range(LH):
                ps = pss.tile([P, LHS], F32, tag="s")
                for j in range(LHS // 512):
                    lo = half * LHS + j * 512
                    for k in range(KT):
                        nc.tensor.matmul(
                            ps[:, j * 512 : (j + 1) * 512],
                            lhsT=wts[pi][k][:, ct * P : (ct + 1) * P],
                            rhs=xt[k][:, lo : lo + 512],
                            start=(k == 0),
                            stop=(k == KT - 1),
                        )
                nc.scalar.activation(
                    dst[:, half * LHS : (half + 1) * LHS],
                    ps,
                    mybir.ActivationFunctionType.Identity,
                    bias=bqk_sb[:, 2 * pi + ct : 2 * pi + ct + 1],
                    scale=1.0,
                )
            (qwT if pi == 0 else kTt)[ct] = dst

        project_qk(0, 0)
        project_qk(0, 1)
        project_qk(1, 0)
        project_qk(1, 1)

        # v in natural layout: 16 tiles [l(128) x c(256)].
        v_sb = []
        for g in range(4):  # groups of 4 l-tiles share one psum fill
            ps = pss.tile([P, LHS], F32, tag="s")
            for li in range(4):
                lt = g * 4 + li
                for k in range(KT):
                    nc.tensor.matmul(
                        ps[:, li * CPC : (li + 1) * CPC],
                        lhsT=xt[k][:, lt * P : (lt + 1) * P],
                        rhs=wts[2][k][:, :],
                        start=(k == 0),
                        stop=(k == KT - 1),
                    )
            for li in range(4):
                lt = g * 4 + li
                t = const.tile([P, CPC], BF16, tag=f"v{lt}")
                nc.vector.tensor_add(
                    out=t, in0=ps[:, li * CPC : (li + 1) * CPC], in1=bvb_sb
                )
                v_sb.append(t)

        # ---- attention -------------------------------------------------
        def attention_chunk(h, lo, width):
            """One head's attention for query columns [lo, lo+width)."""
            nj = width // 512
            ps_av = pav.tile([P, width], F32, tag="av")
            ps_d = pden.tile([1, width], F32, tag="d")
            exps = []
            for mt in range(MT):
                ps_s = pss.tile([P, width], F32, tag="s")
                for j in range(nj):
                    nc.tensor.matmul(
                        ps_s[:, j * 512 : (j + 1) * 512],
                        lhsT=kTt[h][:, mt * P : (mt + 1) * P],
                        rhs=qwT[h][:, lo + j * 512 : lo + (j + 1) * 512],
                        start=True,
                        stop=True,
                    )
                e = pexp.tile([P, width], BF16, tag="exps")
                nc.scalar.activation(e, ps_s, mybir.ActivationFunctionType.Exp)
                exps.append(e)
                for j in range(nj):
                    sl = slice(j * 512, (j + 1) * 512)
                    nc.tensor.matmul(
                        ps_av[:, sl],
                        lhsT=v_sb[mt][:, h * P : (h + 1) * P],
                        rhs=e[:, sl],
                        start=(mt == 0),
                        stop=(mt == MT - 1),
                    )
                    nc.tensor.matmul(
                        ps_d[:, sl],
                        lhsT=ones_col,
                        rhs=e[:, sl],
                        start=(mt == 0),
                        stop=(mt == MT - 1),
                    )

            # evict AV accumulator early so the next chunk's psum slot
            # frees without waiting on the reciprocal chain
            avraw = sb.tile([P, width], F32, tag="avraw")
            nc.vector.tensor_copy(out=avraw, in_=ps_av)

            # reciprocal of denominator: bounce the [1, width] row through
            # DRAM into a [128, width/128] layout so DVE's 8-cycle/elem
            # divide runs on 128 partitions instead of one
            drow = sb.tile([1, width], F32, tag="drow")
            nc.vector.tensor_copy(out=drow, in_=ps_d)
            den_d = pdram.tile([1, width], F32, tag="den_d")
            nc.sync.dma_start(out=den_d, in_=drow)
            w_p = width // P
            dcol = sb.tile([P, w_p], F32, tag="dcol")
            nc.sync.dma_start(
                out=dcol,
                in_=bass.AP(
                    tensor=den_d.tensor,
                    offset=den_d.offset,
                    ap=[[w_p, P], [1, w_p]],
                ),
            )
            rcol = sb.tile([P, w_p], F32, tag="rcol")
            nc.vector.reciprocal(rcol, dcol)
            rcol16 = sb.tile([P, w_p], BF16, tag="rcol16")
            nc.vector.tensor_copy(out=rcol16, in_=rcol)
            recip_d = pdram.tile([1, width], BF16, tag="recip_d")
            nc.sync.dma_start(
                out=bass.AP(
                    tensor=recip_d.tensor,
                    offset=recip_d.offset,
                    ap=[[w_p, P], [1, w_p]],
                ),
                in_=rcol16,
            )
            rb16 = sb.tile([P, width], BF16, tag="rb16")
            nc.sync.dma_start(
                out=rb16,
                in_=bass.AP(
                    tensor=recip_d.tensor,
                    offset=recip_d.offset,
                    ap=[[0, P], [1, width]],
                ),
            )

            # out: normalize and write [c, l] (host transposes)
            outT = sb.tile([P, width], F32, tag="outT")
            nc.vector.tensor_mul(out=outT, in0=avraw, in1=rb16)
            nc.sync.dma_start(out=outpT[h, :, lo : lo + width], in_=outT)

            # attention probabilities: normalize and write [m, l] slab
            for mt in range(MT):
                a = pattn.tile([P, width], BF16, tag="attn")
                nc.vector.tensor_mul(out=a, in0=exps[mt], in1=rb16)
                eng = nc.gpsimd if mt % 2 else nc.sync
                eng.dma_start(
                    out=attnT[h, mt * P : (mt + 1) * P, lo : lo + width],
                    in_=a,
                )

        attention_chunk(0, 0, LHS)
        attention_chunk(1, 0, LHS)
        attention_chunk(0, LHS, LHS)
        attention_chunk(1, LHS, LHS)
    nc.finalize()
    return nc


_PROGRAM_CACHE: dict[str, bass.Bass] = {}


def _get_program() -> bass.Bass:
    if "nc" not in _PROGRAM_CACHE:
        _PROGRAM_CACHE["nc"] = build_program()
    return _PROGRAM_CACHE["nc"]


def make_in_maps(query, Wq, bq, Wk, bk, Wv, bv):
    """Build the 8 per-core input dicts (host-side shard prep)."""
    scale = 1.0 / math.sqrt(D * NB)
    signs = np.tile(REV_SIGNS, DM // NB).astype(np.float32)  # per channel
    bf = ml_dtypes.bfloat16

    in_maps = []
    for c in range(NCORES):
        b = c // 4
        cs = (c % 4) * CPC  # channel start (head-pair offset)
        sl = slice(cs, cs + CPC)
        sgn = (signs[sl] * scale)[:, None]  # (256, 1)

        xT = np.ascontiguousarray(query[b].T).astype(bf)  # (1024, 2048)
        wq = (Wq[sl, :] * sgn).T
        wk = Wk[sl, :].T
        wv = Wv[sl, :].T
        wall = np.ascontiguousarray(
            np.concatenate([wq, wk, wv], axis=1)
        ).astype(bf)  # (1024, 768)
        bq_f = (bq[sl] * sgn[:, 0]).astype(np.float32)
        bk_f = bk[sl].astype(np.float32)
        bqk_arr = np.stack(
            [bq_f[:P], bq_f[P:], bk_f[:P], bk_f[P:]], axis=1
        ).astype(np.float32)  # (128, 4)
        bvb_arr = np.broadcast_to(bv[sl].astype(np.float32), (P, CPC)).copy()

        in_maps.append(dict(xT=xT, wAll=wall, bqk=bqk_arr, bvb=bvb_arr))
    return in_maps


def assemble(results):
    """Gather per-core outputs into (out, attn_mean)."""
    out = np.zeros((B, L, DM), dtype=np.float32)
    attn_sum = np.zeros((B, L, L), dtype=np.float32)
    for c in range(NCORES):
        b = c // 4
        cs = (c % 4) * CPC
        r = results[c]
        ot = np.asarray(r["outpT"], dtype=np.float32)  # (2, 128, L)
        out[b, :, cs : cs + P] = ot[0].T
        out[b, :, cs + P : cs + CPC] = ot[1].T
        at = np.asarray(r["attnT"], dtype=np.float32)  # (2, L(m), L(l))
        attn_sum[b] += at[0].T
        attn_sum[b] += at[1].T
    attn_mean = attn_sum / float(H)
    return out, attn_mean


def run_on_device(in_maps, trace=False):
    from concourse.bass_utils import run_bass_kernel_spmd

    nc = _get_program()
    res = run_bass_kernel_spmd(
        nc, in_maps, core_ids=list(range(NCORES)), trace=trace
    )
    return res


def kernel(query, Wq, bq, Wk, bk, Wv, bv):
    query = np.asarray(query, dtype=np.float32)
    in_maps = make_in_maps(
        query,
        np.asarray(Wq, np.float32),
        np.asarray(bq, np.float32),
        np.asarray(Wk, np.float32),
        np.asarray(bk, np.float32),
        np.asarray(Wv, np.float32),
        np.asarray(bv, np.float32),
    )
    res = run_on_device(in_maps)
    return assemble(res.results)


# revision 40
# speedup vs baseline: 1.0238x; 1.0076x over previous
"""CliffordAttention Trainium2 kernel.

Math (see reference): per (b, h):
    q = x @ Wq.T + bq ; k = x @ Wk.T + bk ; v = x @ Wv.T + bv   (head h owns
    128 contiguous channels of the 1024 output features)
    S[l, m]  = sum_c (q[l,c] * sign_c * scale) * k[m,c]
    attn     = softmax_m(S)
    out[l,c] = sum_m attn[l, m] v[m, c]
Outputs: out (B, L, 1024) and attn.mean over heads (B, L, L).

Sharding: 8 cores = 2 batches x 4 head-pairs. Each core computes its two
heads end-to-end; host sums the per-head attention partials (4 cores x 2
heads per batch), transposes, and divides by 8.

On-device layout: everything transposed (channels on partitions):
    qwT/kT: [c=128, l=2048] per head (sign*scale folded into Wq on host)
    v:      [l=128-tiles, c=256] natural (used as AV lhsT)
    S^T:    [m, l] per (head, l-half) -> exp on ACT -> bf16
    AV:     psum[c, l] = sum_m v[m,c] * expS^T[m,l]
    denom:  psum[1, l] = sum_m expS^T[m,l]   (ones-matmul)
    attn^T: expS^T * bcast(1/denom)  (DVE), written [m, l]; host transposes.
    out:    (psum_av * bcast(1/denom)) -> PE transpose -> [l, c] -> HBM
"""

import math
from contextlib import ExitStack

import numpy as np
import ml_dtypes

import concourse.bass as bass
import concourse.bacc as bacc
import concourse.mybir as mybir
import concourse.tile as tile

B, L, H, D, NB = 2, 2048, 8, 16, 8
DM = H * D * NB  # 1024
P = 128
HPC = 2  # heads per core
CPC = HPC * P  # channels per core (256)
NCORES = 8
KT = DM // P  # 8 K-tiles for projections
MT = L // P  # 16 m-tiles
LH = 2  # l-halves
LHS = L // LH  # 1024

BF16 = mybir.dt.bfloat16
F32 = mybir.dt.float32

REV_SIGNS = np.array([1, 1, 1, 1, -1, -1, -1, -1], dtype=np.float32)


def build_program() -> bass.Bass:
    nc = bacc.Bacc(None, target_bir_lowering=False)

    xT = nc.dram_tensor("xT", [DM, L], BF16, kind="ExternalInput")
    wAll = nc.dram_tensor("wAll", [DM, 3 * CPC], BF16, kind="ExternalInput")
    bqk = nc.dram_tensor("bqk", [P, 4], F32, kind="ExternalInput")
    bvb = nc.dram_tensor("bvb", [P, CPC], F32, kind="ExternalInput")

    attnT = nc.dram_tensor("attnT", [HPC, L, L], BF16, kind="ExternalOutput")
    outpT = nc.dram_tensor("outpT", [HPC, P, L], F32, kind="ExternalOutput")

    with tile.TileContext(nc) as tc, ExitStack() as ctx:
        const = ctx.enter_context(tc.tile_pool(name="const", bufs=1))
        sb = ctx.enter_context(tc.tile_pool(name="sb", bufs=2))
        pexp = ctx.enter_context(tc.tile_pool(name="pexp", bufs=32))
        pattn = ctx.enter_context(tc.tile_pool(name="pattn", bufs=10))
        # PSUM budget is 8 banks of 2KB:
        #   s-tag  [128,1024]f32 x2 bufs = 4 banks (S^T fills, projections,
        #                                           out transposes)
        #   av-tag [128,1024]f32 x1      = 2 banks (AV accumulator, v proj)
        #   d-tag  [1,1024]f32   x1      = 2 banks (denominator row)
        pss = ctx.enter_context(tc.tile_pool(name="pss", bufs=2, space="PSUM"))
        pav = ctx.enter_context(tc.tile_pool(name="pav", bufs=1, space="PSUM"))
        pden = ctx.enter_context(tc.tile_pool(name="pden", bufs=1, space="PSUM"))
        pdram = ctx.enter_context(tc.tile_pool(name="pdram", bufs=2, space="DRAM"))

        # ---- constants -------------------------------------------------
        ones_col = const.tile([P, 1], BF16, tag="ones")
        nc.vector.memset(ones_col, 1.0)
        # warm the tensor engine clock while input DMAs stream in
        scratch = const.tile([P, 512], BF16, tag="scratch")
        nc.vector.memset(scratch, 0.0)
        warm = pden.tile([1, 512], F32, tag="d")
        for _ in range(30):
            nc.tensor.matmul(warm, lhsT=ones_col, rhs=scratch, start=True, stop=True)
        # ---- load inputs: one DMA per K-tile for x and for packed W ----
        xt = []
        wts = [[], [], []]  # wts[p][k] : [128, 256] views into packed tiles
        for k in range(KT):
            tw = const.tile([P, 3 * CPC], BF16, tag=f"w{k}")
            nc.sync.dma_start(out=tw, in_=wAll[k * P : (k + 1) * P, :])
            for pi in range(3):
                wts[pi].append(tw[:, pi * CPC : (pi + 1) * CPC])
            t = const.tile([P, L], BF16, tag=f"xt{k}")
            nc.sync.dma_start(out=t, in_=xT[k * P : (k + 1) * P, :])
            xt.append(t)

        bqk_sb = const.tile([P, 4], F32, tag="bqk")
        nc.sync.dma_start(out=bqk_sb, in_=bqk[:, :])
        bvb_sb = const.tile([P, CPC], F32, tag="bvb")
        nc.sync.dma_start(out=bvb_sb, in_=bvb[:, :])

        # ---- projections ----------------------------------------------
        # q and k in transposed layout: [c(128) x l(2048)] per head.
        qwT = [None, None]
        kTt = [None, None]

        def project_qk(pi, ct):
            dst = const.tile([P, L], BF16, tag=f"p{pi}_{ct}")
            for half in range(LH):
                ps = pss.tile([P, LHS], F32, tag="s")
                for j in range(LHS // 512):
                    lo = half * LHS + j * 512
                    for k in range(KT):
                        nc.tensor.matmul(
                            ps[:, j * 512 : (j + 1) * 512],
                            lhsT=wts[pi][k][:, ct * P : (ct + 1) * P],
                            rhs=xt[k][:, lo : lo + 512],
                            start=(k == 0),
                            stop=(k == KT - 1),
                        )
                nc.scalar.activation(
                    dst[:, half * LHS : (half + 1) * LHS],
                    ps,
                    mybir.ActivationFunctionType.Identity,
                    bias=bqk_sb[:, 2 * pi + ct : 2 * pi + ct + 1],
                    scale=1.0,
                )
            (qwT if pi == 0 else kTt)[ct] = dst

        project_qk(0, 0)
        project_qk(0, 1)
        project_qk(1, 0)
        project_qk(1, 1)

        # v in natural layout: 16 tiles [l(128) x c(256)].
        v_sb = []
        for g in range(4):  # groups of 4 l-tiles share one psum fill
            ps = pss.tile([P, LHS], F32, tag="s")
            for li in range(4):
                lt = g * 4 + li
                for k in range(KT):
                    nc.tensor.matmul(
                        ps[:, li * CPC : (li + 1) * CPC],
                        lhsT=xt[k][:, lt * P : (lt + 1) * P],
                        rhs=wts[2][k][:, :],
                        start=(k == 0),
                        stop=(k == KT - 1),
                    )
            for li in range(4):
                lt = g * 4 + li
                t = const.tile([P, CPC], BF16, tag=f"v{lt}")
                nc.vector.tensor_add(
                    out=t, in0=ps[:, li * CPC : (li + 1) * CPC], in1=bvb_sb
                )
                v_sb.append(t)

        # ---- attention -------------------------------------------------
        def attention_chunk(h, lo, width):
            """One head's attention for query columns [lo, lo+width)."""
            nj = width // 512
            ps_av = pav.tile([P, width], F32, tag="av")
            ps_d = pden.tile([1, width], F32, tag="d")
            exps = []
            for mt in range(MT):
                ps_s = pss.tile([P, width], F32, tag="s")
                for j in range(nj):
                    nc.tensor.matmul(
                        ps_s[:, j * 512 : (j + 1) * 512],
                        lhsT=kTt[h][:, mt * P : (mt + 1) * P],
                        rhs=qwT[h][:, lo + j * 512 : lo + (j + 1) * 512],
                        start=True,
                        stop=True,
                    )
                e = pexp.tile([P, width], BF16, tag="exps")
                nc.scalar.activation(e, ps_s, mybir.ActivationFunctionType.Exp)
                exps.append(e)
                for j in range(nj):
                    sl = slice(j * 512, (j + 1) * 512)
                    nc.tensor.matmul(
                        ps_av[:, sl],
                        lhsT=v_sb[mt][:, h * P : (h + 1) * P],
                        rhs=e[:, sl],
                        start=(mt == 0),
                        stop=(mt == MT - 1),
                    )
                    nc.tensor.matmul(
                        ps_d[:, sl],
                        lhsT=ones_col,
                        rhs=e[:, sl],
                        start=(mt == 0),
                        stop=(mt == MT - 1),
                    )

            # evict AV accumulator early so the next chunk's psum slot
            # frees without waiting on the reciprocal chain
            avraw = sb.tile([P, width], F32, tag="avraw")
            nc.vector.tensor_copy(out=avraw, in_=ps_av)

            # reciprocal of denominator: bounce the [1, width] row through
            # DRAM into a [128, width/128] layout so DVE's 8-cycle/elem
            # divide runs on 128 partitions instead of one
            drow = sb.tile([1, width], F32, tag="drow")
            nc.vector.tensor_copy(out=drow, in_=ps_d)
            den_d = pdram.tile([1, width], F32, tag="den_d")
            nc.sync.dma_start(out=den_d, in_=drow)
            w_p = width // P
            dcol = sb.tile([P, w_p], F32, tag="dcol")
            nc.sync.dma_start(
                out=dcol,
                in_=bass.AP(
                    tensor=den_d.tensor,
                    offset=den_d.offset,
                    ap=[[w_p, P], [1, w_p]],
                ),
            )
            rcol = sb.tile([P, w_p], F32, tag="rcol")
            nc.vector.reciprocal(rcol, dcol)
            rcol16 = sb.tile([P, w_p], BF16, tag="rcol16")
            nc.vector.tensor_copy(out=rcol16, in_=rcol)
            recip_d = pdram.tile([1, width], BF16, tag="recip_d")
            nc.sync.dma_start(
                out=bass.AP(
                    tensor=recip_d.tensor,
                    offset=recip_d.offset,
                    ap=[[w_p, P], [1, w_p]],
                ),
                in_=rcol16,
            )
            rb16 = sb.tile([P, width], BF16, tag="rb16")
            nc.sync.dma_start(
                out=rb16,
                in_=bass.AP(
                    tensor=recip_d.tensor,
                    offset=recip_d.offset,
                    ap=[[0, P], [1, width]],
                ),
            )

            # attention probabilities: normalize in place and write [m, l]
            for mt in range(MT):
                nc.vector.tensor_mul(out=exps[mt], in0=exps[mt], in1=rb16)
                eng = nc.gpsimd if mt % 2 else nc.sync
                eng.dma_start(
                    out=attnT[h, mt * P : (mt + 1) * P, lo : lo + width],
                    in_=exps[mt],
                )

            # out: normalize and write [c, l] (host transposes)
            outT = sb.tile([P, width], F32, tag="outT")
            nc.vector.tensor_mul(out=outT, in0=avraw, in1=rb16)
            nc.sync.dma_start(out=outpT[h, :, lo : lo + width], in_=outT)

        attention_chunk(0, 0, LHS)
        attention_chunk(1, 0, LHS)
        attention_chunk(0, LHS, LHS)
        attention_chunk(1, LHS, LHS)
    nc.finalize()
    return nc


_PROGRAM_CACHE: dict[str, bass.Bass] = {}


def _get_program() -> bass.Bass:
    if "nc" not in _PROGRAM_CACHE:
        _PROGRAM_CACHE["nc"] = build_program()
    return _PROGRAM_CACHE["nc"]


def make_in_maps(query, Wq, bq, Wk, bk, Wv, bv):
    """Build the 8 per-core input dicts (host-side shard prep)."""
    scale = 1.0 / math.sqrt(D * NB)
    signs = np.tile(REV_SIGNS, DM // NB).astype(np.float32)  # per channel
    bf = ml_dtypes.bfloat16

    in_maps = []
    for c in range(NCORES):
        b = c // 4
        cs = (c % 4) * CPC  # channel start (head-pair offset)
        sl = slice(cs, cs + CPC)
        sgn = (signs[sl] * scale)[:, None]  # (256, 1)

        xT = np.ascontiguousarray(query[b].T).astype(bf)  # (1024, 2048)
        wq = (Wq[sl, :] * sgn).T
        wk = Wk[sl, :].T
        wv = Wv[sl, :].T
        wall = np.ascontiguousarray(
            np.concatenate([wq, wk, wv], axis=1)
        ).astype(bf)  # (1024, 768)
        bq_f = (bq[sl] * sgn[:, 0]).astype(np.float32)
        bk_f = bk[sl].astype(np.float32)
        bqk_arr = np.stack(
            [bq_f[:P], bq_f[P:], bk_f[:P], bk_f[P:]], axis=1
        ).astype(np.float32)  # (128, 4)
        bvb_arr = np.broadcast_to(bv[sl].astype(np.float32), (P, CPC)).copy()

        in_maps.append(dict(xT=xT, wAll=wall, bqk=bqk_arr, bvb=bvb_arr))
    return in_maps


def assemble(results):
    """Gather per-core outputs into (out, attn_mean)."""
    out = np.zeros((B, L, DM), dtype=np.float32)
    attn_sum = np.zeros((B, L, L), dtype=np.float32)
    for c in range(NCORES):
        b = c // 4
        cs = (c % 4) * CPC
        r = results[c]
        ot = np.asarray(r["outpT"], dtype=np.float32)  # (2, 128, L)
        out[b, :, cs : cs + P] = ot[0].T
        out[b, :, cs + P : cs + CPC] = ot[1].T
        at = np.asarray(r["attnT"], dtype=np.float32)  # (2, L(m), L(l))
        attn_sum[b] += at[0].T
        attn_sum[b] += at[1].T
    attn_mean = attn_sum / float(H)
    return out, attn_mean


def run_on_device(in_maps, trace=False):
    from concourse.bass_utils import run_bass_kernel_spmd

    nc = _get_program()
    res = run_bass_kernel_spmd(
        nc, in_maps, core_ids=list(range(NCORES)), trace=trace
    )
    return res


def kernel(query, Wq, bq, Wk, bk, Wv, bv):
    query = np.asarray(query, dtype=np.float32)
    in_maps = make_in_maps(
        query,
        np.asarray(Wq, np.float32),
        np.asarray(bq, np.float32),
        np.asarray(Wk, np.float32),
        np.asarray(bk, np.float32),
        np.asarray(Wv, np.float32),
        np.asarray(bv, np.float32),
    )
    res = run_on_device(in_maps)
    return assemble(res.results)
